# revision 9
# baseline (speedup 1.0000x reference)
"""RBF/KNN interpolation kernel for Trainium2 (8 NeuronCores, data parallel).

Computes, per batch b:
    v        = input_data[b, -1, :, 0]                      (N_in,)
    w[o, i]  = exp(-||tc[o] - ic[i]||^2 / (2 * 0.1^2))      (N_out, N_in)
    interp   = (w @ v) / (w.sum(-1) + 1e-8)                 (N_out,)
    out[b]   = broadcast(interp) -> (n_samples, N_out, 4)

Sharding: batch B=8 across 8 cores (one batch per core). The weight matrix
is built on-chip tile by tile (never materialized in HBM):
  - logits  psum[i, o] = icx*tcx + icy*tcy - 0.5*t2[o]      (K=3 matmul)
  - w = Exp(100 * logits + (-50 * i2[i]))                   (scalar engine, bias/scale)
  - [num; den] += [v; 1].T @ w                              (K=128 matmul, psum accum)
"""

import os
from contextlib import ExitStack
from functools import lru_cache

import numpy as np

import concourse.bass as bass
import concourse.bacc as bacc
import concourse.tile as tile
from concourse import mybir
from concourse.bass_utils import run_bass_kernel_spmd

F32 = mybir.dt.float32
AF = mybir.ActivationFunctionType

# Problem sizes (hardcoded per spec)
B = 8
T_IN = 4
N_IN = 4096
V_IN = 3
N_OUT = 8192
S = 10
T_OUT = 4
GAMMA = 50.0  # 1 / (2 * LENGTH_SCALE^2), LENGTH_SCALE = 0.1
EPS = 1e-8


def build_kernel(tc_ctx, dat, ic_h, tc_h, out_h, n_in, n_out, s, F=1024):
    """Emit the per-core kernel into TileContext tc_ctx.

    dat:  [T_IN, n_in, V_IN] f32   (only [-1, :, 0] is used)
    ic_h: [n_in, 2]  f32
    tc_h: [n_out, 2] f32
    out_h: [s, n_out, 4] f32
    """
    tcx = tc_ctx
    nc = tcx.nc
    IC = n_in // 128   # number of i-chunks
    OC = n_out // F    # number of o-chunks
    NSUB = F // 512    # 512-wide sub-chunks per o-chunk
    L = n_out // 128   # per-partition interp count in the output layout

    with ExitStack() as ctx:
        const_pool = ctx.enter_context(tcx.tile_pool(name="const", bufs=1))

        # ---- persistent tiles ----
        tc_aug = const_pool.tile([3, n_out], F32)    # rows: -0.5*t2, tcx, tcy
        ic_aug = const_pool.tile([3, n_in], F32)     # rows: ones, icx, icy
        tc_nat = const_pool.tile([128, 2 * (n_out // 128)], F32)
        ic_nat = const_pool.tile([128, 2 * IC], F32)
        sq_t = const_pool.tile([128, 2 * (n_out // 128)], F32)
        sq_i = const_pool.tile([128, 2 * IC], F32)
        t2m_nat = const_pool.tile([128, n_out // 128], F32)
        i2_nat = const_pool.tile([128, IC], F32)
        bias_nat = const_pool.tile([128, IC], F32)   # -50 * i2, chunk-major
        # lhsT for num/den: per chunk 33 cols = [v, 0 x31, 1]; den lands on
        # psum partition 32 (compute-engine PSUM APs must be 32-aligned)
        vo_nat = const_pool.tile([128, 33 * IC], F32)
        v_nat = const_pool.tile([128, IC], F32)
        interp_row = const_pool.tile([1, n_out], F32)

        # ---- input DMAs (strided layout loads) ----
        # coord-major coords: tc_aug[0:2, o] = tc[o, :].T ; ic_aug[0:2, i]
        nsp = 4
        for q in range(nsp):
            c0, c1 = q * (n_out // nsp), (q + 1) * (n_out // nsp)
            nc.sync.dma_start(
                out=tc_aug[1:3, c0:c1],
                in_=tc_h[:].rearrange("o d -> d o")[:, c0:c1],
            )
        for q in range(2):
            c0, c1 = q * (n_in // 2), (q + 1) * (n_in // 2)
            nc.sync.dma_start(
                out=ic_aug[1:3, c0:c1],
                in_=ic_h[:].rearrange("i d -> d i")[:, c0:c1],
            )
        # natural chunk-major layouts: x[p, 2c+d] = coords[c*128 + p, d]
        nc.sync.dma_start(
            out=tc_nat.rearrange("p (c two) -> p c two", two=2),
            in_=tc_h[:].rearrange("(c p) d -> p c d", p=128),
        )
        nc.sync.dma_start(
            out=ic_nat.rearrange("p (c two) -> p c two", two=2),
            in_=ic_h[:].rearrange("(c p) d -> p c d", p=128),
        )
        # values: v[p, c] = input_data[-1, c*128 + p, 0]
        nc.sync.dma_start(
            out=v_nat[:, :],
            in_=dat[:][T_IN - 1, :, 0].rearrange("(c p) -> p c", p=128),
        )

        # ---- small preprocessing ----
        # ones row for lhsT
        nc.vector.memset(ic_aug[0:1, :], 1.0)

        # t2 = tcx^2 + tcy^2 (chunk-major), scaled by -0.5
        nc.vector.tensor_mul(sq_t[:, :], tc_nat[:, :], tc_nat[:, :])
        sq_t3 = sq_t.rearrange("p (c two) -> p c two", two=2)
        nc.vector.tensor_add(t2m_nat[:, :], sq_t3[:, :, 0], sq_t3[:, :, 1])
        nc.vector.tensor_scalar_mul(t2m_nat[:, :], t2m_nat[:, :], -0.5)
        # scatter chunk-major -> row-major: tc_aug[0, c*128 + p] = t2m_nat[p, c]
        for c in range(n_out // 128):
            nc.sync.dma_start(
                out=tc_aug[0:1, c * 128:(c + 1) * 128],
                in_=t2m_nat[:, c:c + 1],
            )

        # i2 = icx^2 + icy^2 (chunk-major), bias = -50 * i2
        nc.vector.tensor_mul(sq_i[:, :], ic_nat[:, :], ic_nat[:, :])
        sq_i3 = sq_i.rearrange("p (c two) -> p c two", two=2)
        nc.vector.tensor_add(i2_nat[:, :], sq_i3[:, :, 0], sq_i3[:, :, 1])
        nc.vector.tensor_scalar_mul(bias_nat[:, :], i2_nat[:, :], -GAMMA)

        # vo_nat: col 33c = v[:, c], col 33c+32 = 1.0, rest 0
        nc.gpsimd.memset(vo_nat[:, :], 0.0)
        vo3 = vo_nat.rearrange("p (c w) -> p c w", w=33)
        nc.vector.tensor_copy(vo3[:, :, 0], v_nat[:, :])
        nc.vector.memset(vo3[:, :, 32], 1.0)

        # ---- main loop ----
        with (
            tcx.tile_pool(name="psum_l", bufs=2, space="PSUM") as pl_pool,
            tcx.tile_pool(name="psum_nd", bufs=2, space="PSUM") as nd_pool,
            tcx.tile_pool(name="w", bufs=3) as w_pool,
            tcx.tile_pool(name="div", bufs=2) as div_pool,
        ):
            for oc in range(OC):
                nd = nd_pool.tile([33, F], F32)
                for icc in range(IC):
                    pl = pl_pool.tile([128, F], F32)
                    lhsT1 = ic_aug[0:3, icc * 128:(icc + 1) * 128]
                    for sub in range(NSUB):
                        nc.tensor.matmul(
                            pl[:, sub * 512:(sub + 1) * 512],
                            lhsT1,
                            tc_aug[0:3, oc * F + sub * 512: oc * F + (sub + 1) * 512],
                            start=True,
                            stop=True,
                        )
                    w = w_pool.tile([128, F], F32)
                    nc.scalar.activation(
                        w[:, :],
                        pl[:, :],
                        AF.Exp,
                        bias=bias_nat[:, icc:icc + 1],
                        scale=2.0 * GAMMA,
                    )
                    for sub in range(NSUB):
                        nc.tensor.matmul(
                            nd[:, sub * 512:(sub + 1) * 512],
                            vo_nat[:, 33 * icc:33 * icc + 33],
                            w[:, sub * 512:(sub + 1) * 512],
                            start=(icc == 0),
                            stop=(icc == IC - 1),
                        )
                # interp = num / (den + eps)
                den_eps = div_pool.tile([1, F], F32, tag="den")
                recip = div_pool.tile([1, F], F32, tag="recip")
                nc.vector.tensor_scalar_add(den_eps[:, :], nd[32:33, :], EPS)
                nc.vector.reciprocal(recip[:, :], den_eps[:, :])
                nc.vector.tensor_mul(
                    interp_row[0:1, oc * F:(oc + 1) * F], nd[0:1, :], recip[:, :]
                )

        # ---- output broadcast: out[s, o, t] = interp[o] ----
        with tcx.tile_pool(name="tail", bufs=1) as tail_pool:
            # p-major layout: interp_pm[p, k] = interp[p*L + k]
            interp_pm = tail_pool.tile([128, L], F32)
            nc.sync.dma_start(
                out=interp_pm[:, :],
                in_=interp_row[0:1, :].rearrange("r (p k) -> r p k", p=128),
            )
            # replicate x4 along t: rep[p, 4k + t] = interp_pm[p, k]
            rep_sb = tail_pool.tile([128, 4 * L], F32)
            rep3 = rep_sb.rearrange("p (k t) -> p k t", t=4)
            for t in range(4):
                nc.vector.tensor_copy(rep3[:, :, t], interp_pm[:, :])
            # each s-copy is one contiguous 128 x (4L) x 4B block
            for si in range(s):
                nc.sync.dma_start(
                    out=out_h[:][si].rearrange("o t -> (o t)").rearrange(
                        "(p j) -> p j", p=128),
                    in_=rep_sb[:, :],
                )


@lru_cache(maxsize=2)
def build_nc(n_in=N_IN, n_out=N_OUT, s=S, F=1024):
    nc = bacc.Bacc("TRN2", target_bir_lowering=False, debug=False)
    dat = nc.dram_tensor("dat", [T_IN, n_in, V_IN], F32, kind="ExternalInput")
    ic_h = nc.dram_tensor("ic", [n_in, 2], F32, kind="ExternalInput")
    tc_h = nc.dram_tensor("tc", [n_out, 2], F32, kind="ExternalInput")
    out_h = nc.dram_tensor("out", [s, n_out, T_OUT], F32, kind="ExternalOutput")
    with tile.TileContext(nc) as tcx:
        build_kernel(tcx, dat, ic_h, tc_h, out_h, n_in, n_out, s, F=F)
    nc.compile()
    return nc


def _run(input_data, input_coords, target_coords, n_samples, trace=False):
    n_samples = int(n_samples)
    assert n_samples == S, f"kernel compiled for n_samples={S}, got {n_samples}"
    assert input_data.shape == (B, T_IN, N_IN, V_IN)
    nc = build_nc()
    in_maps = [
        {
            "dat": np.ascontiguousarray(input_data[b], dtype=np.float32),
            "ic": np.ascontiguousarray(input_coords[b], dtype=np.float32),
            "tc": np.ascontiguousarray(target_coords[b], dtype=np.float32),
        }
        for b in range(B)
    ]
    res = run_bass_kernel_spmd(nc, in_maps, list(range(B)), trace=trace)
    out = np.stack([res.results[b]["out"] for b in range(B)], axis=0)
    return out, res


def kernel(input_data, input_coords, target_coords, n_samples):
    out, _ = _run(
        np.asarray(input_data),
        np.asarray(input_coords),
        np.asarray(target_coords),
        n_samples,
    )
    return out


# revision 11
# speedup vs baseline: 2.3958x; 2.3958x over previous
"""RBF/KNN interpolation kernel for Trainium2 (8 NeuronCores, data parallel).

Computes, per batch b:
    v        = input_data[b, -1, :, 0]                      (N_in,)
    w[o, i]  = exp(-||tc[o] - ic[i]||^2 / (2 * 0.1^2))      (N_out, N_in)
    interp   = (w @ v) / (w.sum(-1) + 1e-8)                 (N_out,)
    out[b]   = broadcast(interp) -> (n_samples, N_out, 4)

Sharding: batch B=8 across 8 cores (one batch per core). The weight matrix
is built on-chip tile by tile (never materialized in HBM):
  - logits psum[i, o] via a K=8 fp16 matmul. fp32 coords are split into
    fp16 (hi, lo) pairs so the single-pass fp16 PE path keeps ~1e-4
    precision on the exponent (fp32 matmul runs 2 passes at half rate):
      cross = xh*txh + xh*txl + xl*txh + (same for y) + 1*t2h + 1*t2l
    where t2h + t2l ~= -0.5 * |tc|^2.
  - w = Exp(100 * logits + bias[i]) on the scalar engine, written as fp16;
    bias = -50*|ic|^2 + 10*ln(2) (the 2^10 factor keeps small weights out
    of the fp16 denormal range; it cancels in num/den).
  - [num; ...; den] += [v, 0 x31, 1].T @ w  (fp16 matmul, fp32 psum accum;
    den lands on psum partition 32 - compute-engine PSUM APs need 32-aligned
    starts).
  - interp = num / (den + 1024e-8), computed in a [128, L] layout, then
    broadcast x4 (vector copies) and x n_samples (DMA) to the output.
"""

from contextlib import ExitStack
from functools import lru_cache

import numpy as np

import concourse.bass as bass
import concourse.bacc as bacc
import concourse.tile as tile
from concourse import mybir
from concourse.bass_utils import run_bass_kernel_spmd

F32 = mybir.dt.float32
F16 = mybir.dt.float16
AF = mybir.ActivationFunctionType
ALU = mybir.AluOpType

# Problem sizes (hardcoded per spec)
B = 8
T_IN = 4
N_IN = 4096
V_IN = 3
N_OUT = 8192
S = 10
T_OUT = 4
GAMMA = 50.0  # 1 / (2 * LENGTH_SCALE^2), LENGTH_SCALE = 0.1
EPS = 1e-8
WSCALE_LOG = 6.93147180559945  # ln(2^10)
WSCALE = 1024.0


def build_kernel(tc_ctx, dat, ic_h, tc_h, out_h, n_in, n_out, s, F=1024):
    tcx = tc_ctx
    nc = tcx.nc
    IC = n_in // 128   # i-chunks
    OC = n_out // F    # o-chunks
    NSUB = F // 512
    L = n_out // 128   # per-partition interp count in output layout
    CT = n_out // 128  # nat-layout columns (target side)

    with ExitStack() as ctx:
        const_pool = ctx.enter_context(tcx.tile_pool(name="const", bufs=1))

        # ---- persistent tiles ----
        tc_aug = const_pool.tile([8, n_out], F16)  # rows t2h t2l txh txl txh tyh tyl tyh
        ic_aug = const_pool.tile([8, n_in], F16)   # rows 1   1   xh  xh  xl  yh  yh  yl
        bias_nat = const_pool.tile([128, IC], F32)
        vo_nat = const_pool.tile([128, 33 * IC], F16)  # [v, 0*31, 1] per chunk
        num_row = const_pool.tile([1, n_out], F32)
        den_row = const_pool.tile([1, n_out], F32)
        ident = const_pool.tile([128, 128], F16)

        # ---- head: inputs, identity, splits (all in 128-partition nat layout) ----
        head = ctx.enter_context(tcx.tile_pool(name="head", bufs=1))
        tc_nat = head.tile([128, 2 * CT], F32)
        ic_nat = head.tile([128, 2 * IC], F32)
        v_nat = head.tile([128, IC], F32)

        nc.sync.dma_start(
            out=tc_nat.rearrange("p (c two) -> p c two", two=2),
            in_=tc_h[:].rearrange("(c p) d -> p c d", p=128),
        )
        nc.sync.dma_start(
            out=ic_nat.rearrange("p (c two) -> p c two", two=2),
            in_=ic_h[:].rearrange("(c p) d -> p c d", p=128),
        )
        nc.sync.dma_start(
            out=v_nat[:, :],
            in_=dat[:][T_IN - 1, :, 0].rearrange("(c p) -> p c", p=128),
        )

        # identity for PE transposes: ident[p, f] = (p == f)
        jj = head.tile([128, 128], F32)
        kk = head.tile([128, 1], F32)
        nc.gpsimd.iota(jj[:, :], [[1, 128]], base=0, channel_multiplier=0,
                       allow_small_or_imprecise_dtypes=True)
        nc.gpsimd.iota(kk[:, :], [[0, 1]], base=0, channel_multiplier=1,
                       allow_small_or_imprecise_dtypes=True)
        nc.vector.tensor_scalar(ident[:, :], jj[:, :], kk[:, 0:1], None,
                                op0=ALU.is_equal)

        # --- target-side nat computes (chunk-major: x[p, c] = f(tc[c*128+p])) ---
        tc3 = tc_nat.rearrange("p (c two) -> p c two", two=2)
        tcx_nat = tc3[:, :, 0]
        tcy_nat = tc3[:, :, 1]
        sq_t = head.tile([128, 2 * CT], F32)
        nc.vector.tensor_mul(sq_t[:, :], tc_nat[:, :], tc_nat[:, :])
        sq_t3 = sq_t.rearrange("p (c two) -> p c two", two=2)
        t2s = head.tile([128, CT], F32)
        nc.vector.tensor_add(t2s[:, :], sq_t3[:, :, 0], sq_t3[:, :, 1])

        t2h_nat = head.tile([128, CT], F16)
        t2l_nat = head.tile([128, CT], F16)
        nc.vector.tensor_scalar_mul(t2h_nat[:, :], t2s[:, :], -0.5)
        nc.vector.scalar_tensor_tensor(t2l_nat[:, :], t2s[:, :], -0.5,
                                       t2h_nat[:, :], op0=ALU.mult,
                                       op1=ALU.subtract)
        txh_nat = head.tile([128, CT], F16)
        txl_nat = head.tile([128, CT], F16)
        tyh_nat = head.tile([128, CT], F16)
        tyl_nat = head.tile([128, CT], F16)
        nc.vector.tensor_copy(txh_nat[:, :], tcx_nat)
        nc.vector.tensor_sub(txl_nat[:, :], tcx_nat, txh_nat[:, :])
        nc.vector.tensor_copy(tyh_nat[:, :], tcy_nat)
        nc.vector.tensor_sub(tyl_nat[:, :], tcy_nat, tyh_nat[:, :])

        # --- input-side nat computes ---
        ic3 = ic_nat.rearrange("p (c two) -> p c two", two=2)
        icx_nat = ic3[:, :, 0]
        icy_nat = ic3[:, :, 1]
        sq_i = head.tile([128, 2 * IC], F32)
        nc.vector.tensor_mul(sq_i[:, :], ic_nat[:, :], ic_nat[:, :])
        sq_i3 = sq_i.rearrange("p (c two) -> p c two", two=2)
        i2s = head.tile([128, IC], F32)
        nc.vector.tensor_add(i2s[:, :], sq_i3[:, :, 0], sq_i3[:, :, 1])
        # bias = -50 * i2 + ln(2^10)
        nc.vector.tensor_scalar(bias_nat[:, :], i2s[:, :], -GAMMA, WSCALE_LOG,
                                op0=ALU.mult, op1=ALU.add)

        xh_nat = head.tile([128, IC], F16)
        xl_nat = head.tile([128, IC], F16)
        yh_nat = head.tile([128, IC], F16)
        yl_nat = head.tile([128, IC], F16)
        nc.vector.tensor_copy(xh_nat[:, :], icx_nat)
        nc.vector.tensor_sub(xl_nat[:, :], icx_nat, xh_nat[:, :])
        nc.vector.tensor_copy(yh_nat[:, :], icy_nat)
        nc.vector.tensor_sub(yl_nat[:, :], icy_nat, yh_nat[:, :])

        # vo_nat: col 33c = v (fp16), col 33c+32 = 1.0, rest 0
        nc.gpsimd.memset(vo_nat[:, :], 0.0)
        vo3 = vo_nat.rearrange("p (c w) -> p c w", w=33)
        nc.vector.tensor_copy(vo3[:, :, 0], v_nat[:, :])
        nc.vector.memset(vo3[:, :, 32], 1.0)

        # --- nat -> row layout via PE transpose + copy + DMA ---
        with tcx.tile_pool(name="tps", bufs=2, space="PSUM") as tp_pool, \
             tcx.tile_pool(name="tsb", bufs=2) as tsb_pool:

            def to_rows(nat, ncols, aug, rows):
                ps = tp_pool.tile([128, 128], F16, tag="ps")
                sb = tsb_pool.tile([128, 128], F16, tag="sb")
                nc.tensor.transpose(ps[:ncols, :], nat[:, :], ident[:, :])
                nc.vector.tensor_copy(sb[:ncols, :], ps[:ncols, :])
                for r in rows:
                    nc.sync.dma_start(
                        out=aug[r:r + 1, :].rearrange("r (c p) -> r c p", p=128),
                        in_=sb[:ncols, :],
                    )

            to_rows(t2h_nat, CT, tc_aug, [0])
            to_rows(t2l_nat, CT, tc_aug, [1])
            to_rows(txh_nat, CT, tc_aug, [2, 4])
            to_rows(txl_nat, CT, tc_aug, [3])
            to_rows(tyh_nat, CT, tc_aug, [5, 7])
            to_rows(tyl_nat, CT, tc_aug, [6])

            nc.vector.memset(ic_aug[0:2, :], 1.0)
            to_rows(xh_nat, IC, ic_aug, [2, 3])
            to_rows(xl_nat, IC, ic_aug, [4])
            to_rows(yh_nat, IC, ic_aug, [5, 6])
            to_rows(yl_nat, IC, ic_aug, [7])

        # ---- main loop ----
        with (
            tcx.tile_pool(name="psum_l", bufs=2, space="PSUM") as pl_pool,
            tcx.tile_pool(name="psum_nd", bufs=2, space="PSUM") as nd_pool,
            tcx.tile_pool(name="w", bufs=3) as w_pool,
        ):
            for oc in range(OC):
                nd = nd_pool.tile([33, F], F32)
                for icc in range(IC):
                    pl = pl_pool.tile([128, F], F32)
                    lhsT1 = ic_aug[0:8, icc * 128:(icc + 1) * 128]
                    for sub in range(NSUB):
                        nc.tensor.matmul(
                            pl[:, sub * 512:(sub + 1) * 512],
                            lhsT1,
                            tc_aug[0:8, oc * F + sub * 512: oc * F + (sub + 1) * 512],
                            start=True,
                            stop=True,
                        )
                    w = w_pool.tile([128, F], F16)
                    nc.scalar.activation(
                        w[:, :],
                        pl[:, :],
                        AF.Exp,
                        bias=bias_nat[:, icc:icc + 1],
                        scale=2.0 * GAMMA,
                    )
                    for sub in range(NSUB):
                        nc.tensor.matmul(
                            nd[:, sub * 512:(sub + 1) * 512],
                            vo_nat[:, 33 * icc:33 * icc + 33],
                            w[:, sub * 512:(sub + 1) * 512],
                            start=(icc == 0),
                            stop=(icc == IC - 1),
                        )
                nc.vector.tensor_copy(num_row[0:1, oc * F:(oc + 1) * F], nd[0:1, :])
                nc.vector.tensor_copy(den_row[0:1, oc * F:(oc + 1) * F], nd[32:33, :])

        # ---- tail: divide + broadcast in [128, L] layout ----
        with tcx.tile_pool(name="tail", bufs=1) as tail_pool:
            num_pm = tail_pool.tile([128, L], F32)
            den_pm = tail_pool.tile([128, L], F32)
            nc.sync.dma_start(
                out=num_pm[:, :],
                in_=num_row[0:1, :].rearrange("r (p k) -> r p k", p=128),
            )
            nc.sync.dma_start(
                out=den_pm[:, :],
                in_=den_row[0:1, :].rearrange("r (p k) -> r p k", p=128),
            )
            recip = tail_pool.tile([128, L], F32)
            nc.vector.tensor_scalar_add(recip[:, :], den_pm[:, :], EPS * WSCALE)
            nc.vector.reciprocal(recip[:, :], recip[:, :])
            interp_pm = tail_pool.tile([128, L], F32)
            nc.vector.tensor_mul(interp_pm[:, :], num_pm[:, :], recip[:, :])

            rep_sb = tail_pool.tile([128, 4 * L], F32)
            rep3 = rep_sb.rearrange("p (k t) -> p k t", t=4)
            for t in range(4):
                nc.vector.tensor_copy(rep3[:, :, t], interp_pm[:, :])
            for si in range(s):
                nc.sync.dma_start(
                    out=out_h[:][si].rearrange("o t -> (o t)").rearrange(
                        "(p j) -> p j", p=128),
                    in_=rep_sb[:, :],
                )


@lru_cache(maxsize=2)
def build_nc(n_in=N_IN, n_out=N_OUT, s=S, F=1024):
    nc = bacc.Bacc("TRN2", target_bir_lowering=False, debug=False)
    dat = nc.dram_tensor("dat", [T_IN, n_in, V_IN], F32, kind="ExternalInput")
    ic_h = nc.dram_tensor("ic", [n_in, 2], F32, kind="ExternalInput")
    tc_h = nc.dram_tensor("tc", [n_out, 2], F32, kind="ExternalInput")
    out_h = nc.dram_tensor("out", [s, n_out, T_OUT], F32, kind="ExternalOutput")
    with tile.TileContext(nc) as tcx:
        build_kernel(tcx, dat, ic_h, tc_h, out_h, n_in, n_out, s, F=F)
    nc.compile()
    return nc


def _run(input_data, input_coords, target_coords, n_samples, trace=False):
    n_samples = int(n_samples)
    assert n_samples == S, f"kernel compiled for n_samples={S}, got {n_samples}"
    assert input_data.shape == (B, T_IN, N_IN, V_IN)
    nc = build_nc()
    in_maps = [
        {
            "dat": np.ascontiguousarray(input_data[b], dtype=np.float32),
            "ic": np.ascontiguousarray(input_coords[b], dtype=np.float32),
            "tc": np.ascontiguousarray(target_coords[b], dtype=np.float32),
        }
        for b in range(B)
    ]
    res = run_bass_kernel_spmd(nc, in_maps, list(range(B)), trace=trace)
    out = np.stack([res.results[b]["out"] for b in range(B)], axis=0)
    return out, res


def kernel(input_data, input_coords, target_coords, n_samples):
    out, _ = _run(
        np.asarray(input_data),
        np.asarray(input_coords),
        np.asarray(target_coords),
        n_samples,
    )
    return out


# revision 12
# speedup vs baseline: 3.7228x; 1.5539x over previous
"""RBF/KNN interpolation kernel for Trainium2 (8 NeuronCores, data parallel).

Computes, per batch b:
    v        = input_data[b, -1, :, 0]                      (N_in,)
    w[o, i]  = exp(-||tc[o] - ic[i]||^2 / (2 * 0.1^2))      (N_out, N_in)
    interp   = (w @ v) / (w.sum(-1) + 1e-8)                 (N_out,)
    out[b]   = broadcast(interp) -> (n_samples, N_out, 4)

Sharding: batch B=8 across 8 cores (one batch per core). The weight matrix
is built on-chip tile by tile (never materialized in HBM):
  - logits psum[i, o] via a K=8 fp16 matmul. fp32 coords are split into
    fp16 (hi, lo) pairs so the single-pass fp16 PE path keeps ~1e-4
    precision on the exponent (fp32 matmul runs 2 passes at half rate):
      cross = xh*txh + xh*txl + xl*txh + (same for y) + 1*t2h + 1*t2l
    where t2h + t2l ~= -0.5 * |tc|^2.
  - w = Exp(100 * logits + bias[i]) on the scalar engine, written as fp16;
    bias = -50*|ic|^2 + 10*ln(2) (the 2^10 factor keeps small weights out
    of the fp16 denormal range; it cancels in num/den).
  - [num; ...; den] += [v, 0 x31, 1].T @ w  (fp16 matmul, fp32 psum accum;
    den lands on psum partition 32 - compute-engine PSUM APs need 32-aligned
    starts).
  - interp = num / (den + 1024e-8), computed in a [128, L] layout, then
    broadcast x4 (vector copies) and x n_samples (DMA) to the output.
"""

from contextlib import ExitStack
from functools import lru_cache

import numpy as np

import concourse.bass as bass
import concourse.bacc as bacc
import concourse.tile as tile
from concourse import mybir
from concourse.bass_utils import run_bass_kernel_spmd

F32 = mybir.dt.float32
F16 = mybir.dt.float16
AF = mybir.ActivationFunctionType
ALU = mybir.AluOpType

# Problem sizes (hardcoded per spec)
B = 8
T_IN = 4
N_IN = 4096
V_IN = 3
N_OUT = 8192
S = 10
T_OUT = 4
GAMMA = 50.0  # 1 / (2 * LENGTH_SCALE^2), LENGTH_SCALE = 0.1
EPS = 1e-8
WSCALE_LOG = 6.93147180559945  # ln(2^10)
WSCALE = 1024.0


def build_kernel(tc_ctx, dat, ic_h, tc_h, out_h, n_in, n_out, s, F=1024):
    tcx = tc_ctx
    nc = tcx.nc
    IC = n_in // 128   # i-chunks
    OC = n_out // F    # o-chunks
    NSUB = F // 512
    L = n_out // 128   # per-partition interp count in output layout
    CT = n_out // 128  # nat-layout columns (target side)

    with ExitStack() as ctx:
        const_pool = ctx.enter_context(tcx.tile_pool(name="const", bufs=1))

        # ---- persistent tiles ----
        # K is zero-padded 8 -> 128: a full-array matmul costs the same cycles
        # (stream rate is per column) but keeps the PE HAM activity monitor
        # seeing a busy array, so the clock un-throttles to 2.4 GHz.
        tc_aug = const_pool.tile([128, n_out], F16)  # rows t2h t2l txh txl txh tyh tyl tyh, rest 0
        ic_aug = const_pool.tile([128, n_in], F16)   # rows 1   1   xh  xh  xl  yh  yh  yl, rest 0
        bias_nat = const_pool.tile([128, IC], F32)
        vo_nat = const_pool.tile([128, 128 * IC], F16)  # [v, 0..., 1@32, 0...] per chunk
        num_row = const_pool.tile([1, n_out], F32)
        den_row = const_pool.tile([1, n_out], F32)
        ident = const_pool.tile([128, 128], F16)

        # ---- head: inputs, identity, splits (all in 128-partition nat layout) ----
        head = ctx.enter_context(tcx.tile_pool(name="head", bufs=1))
        tc_nat = head.tile([128, 2 * CT], F32)
        ic_nat = head.tile([128, 2 * IC], F32)
        v_nat = head.tile([128, IC], F32)

        nc.sync.dma_start(
            out=tc_nat.rearrange("p (c two) -> p c two", two=2),
            in_=tc_h[:].rearrange("(c p) d -> p c d", p=128),
        )
        nc.sync.dma_start(
            out=ic_nat.rearrange("p (c two) -> p c two", two=2),
            in_=ic_h[:].rearrange("(c p) d -> p c d", p=128),
        )
        nc.sync.dma_start(
            out=v_nat[:, :],
            in_=dat[:][T_IN - 1, :, 0].rearrange("(c p) -> p c", p=128),
        )

        # identity for PE transposes: ident[p, f] = (p == f)
        jj = head.tile([128, 128], F32)
        kk = head.tile([128, 1], F32)
        nc.gpsimd.iota(jj[:, :], [[1, 128]], base=0, channel_multiplier=0,
                       allow_small_or_imprecise_dtypes=True)
        nc.gpsimd.iota(kk[:, :], [[0, 1]], base=0, channel_multiplier=1,
                       allow_small_or_imprecise_dtypes=True)
        nc.vector.tensor_scalar(ident[:, :], jj[:, :], kk[:, 0:1], None,
                                op0=ALU.is_equal)

        # --- target-side nat computes (chunk-major: x[p, c] = f(tc[c*128+p])) ---
        tc3 = tc_nat.rearrange("p (c two) -> p c two", two=2)
        tcx_nat = tc3[:, :, 0]
        tcy_nat = tc3[:, :, 1]
        sq_t = head.tile([128, 2 * CT], F32)
        nc.vector.tensor_mul(sq_t[:, :], tc_nat[:, :], tc_nat[:, :])
        sq_t3 = sq_t.rearrange("p (c two) -> p c two", two=2)
        t2s = head.tile([128, CT], F32)
        nc.vector.tensor_add(t2s[:, :], sq_t3[:, :, 0], sq_t3[:, :, 1])

        t2h_nat = head.tile([128, CT], F16)
        t2l_nat = head.tile([128, CT], F16)
        nc.vector.tensor_scalar_mul(t2h_nat[:, :], t2s[:, :], -0.5)
        nc.vector.scalar_tensor_tensor(t2l_nat[:, :], t2s[:, :], -0.5,
                                       t2h_nat[:, :], op0=ALU.mult,
                                       op1=ALU.subtract)
        txh_nat = head.tile([128, CT], F16)
        txl_nat = head.tile([128, CT], F16)
        tyh_nat = head.tile([128, CT], F16)
        tyl_nat = head.tile([128, CT], F16)
        nc.vector.tensor_copy(txh_nat[:, :], tcx_nat)
        nc.vector.tensor_sub(txl_nat[:, :], tcx_nat, txh_nat[:, :])
        nc.vector.tensor_copy(tyh_nat[:, :], tcy_nat)
        nc.vector.tensor_sub(tyl_nat[:, :], tcy_nat, tyh_nat[:, :])

        # --- input-side nat computes ---
        ic3 = ic_nat.rearrange("p (c two) -> p c two", two=2)
        icx_nat = ic3[:, :, 0]
        icy_nat = ic3[:, :, 1]
        sq_i = head.tile([128, 2 * IC], F32)
        nc.vector.tensor_mul(sq_i[:, :], ic_nat[:, :], ic_nat[:, :])
        sq_i3 = sq_i.rearrange("p (c two) -> p c two", two=2)
        i2s = head.tile([128, IC], F32)
        nc.vector.tensor_add(i2s[:, :], sq_i3[:, :, 0], sq_i3[:, :, 1])
        # bias = -50 * i2 + ln(2^10)
        nc.vector.tensor_scalar(bias_nat[:, :], i2s[:, :], -GAMMA, WSCALE_LOG,
                                op0=ALU.mult, op1=ALU.add)

        xh_nat = head.tile([128, IC], F16)
        xl_nat = head.tile([128, IC], F16)
        yh_nat = head.tile([128, IC], F16)
        yl_nat = head.tile([128, IC], F16)
        nc.vector.tensor_copy(xh_nat[:, :], icx_nat)
        nc.vector.tensor_sub(xl_nat[:, :], icx_nat, xh_nat[:, :])
        nc.vector.tensor_copy(yh_nat[:, :], icy_nat)
        nc.vector.tensor_sub(yl_nat[:, :], icy_nat, yh_nat[:, :])

        # vo_nat: col 128c = v (fp16), col 128c+32 = 1.0, rest 0 (M padded to 128)
        nc.gpsimd.memset(vo_nat[:, :], 0.0)
        vo3 = vo_nat.rearrange("p (c w) -> p c w", w=128)
        nc.vector.tensor_copy(vo3[:, :, 0], v_nat[:, :])
        nc.vector.memset(vo3[:, :, 32], 1.0)
        # zero the unused K rows of the logit operands once
        nc.gpsimd.memset(tc_aug[:, :], 0.0)
        nc.gpsimd.memset(ic_aug[:, :], 0.0)

        # --- nat -> row layout via PE transpose + copy + DMA ---
        with tcx.tile_pool(name="tps", bufs=2, space="PSUM") as tp_pool, \
             tcx.tile_pool(name="tsb", bufs=2) as tsb_pool:

            def to_rows(nat, ncols, aug, rows):
                ps = tp_pool.tile([128, 128], F16, tag="ps")
                sb = tsb_pool.tile([128, 128], F16, tag="sb")
                nc.tensor.transpose(ps[:ncols, :], nat[:, :], ident[:, :])
                nc.vector.tensor_copy(sb[:ncols, :], ps[:ncols, :])
                for r in rows:
                    nc.sync.dma_start(
                        out=aug[r:r + 1, :].rearrange("r (c p) -> r c p", p=128),
                        in_=sb[:ncols, :],
                    )

            to_rows(t2h_nat, CT, tc_aug, [0])
            to_rows(t2l_nat, CT, tc_aug, [1])
            to_rows(txh_nat, CT, tc_aug, [2, 4])
            to_rows(txl_nat, CT, tc_aug, [3])
            to_rows(tyh_nat, CT, tc_aug, [5, 7])
            to_rows(tyl_nat, CT, tc_aug, [6])

            nc.vector.memset(ic_aug[0:2, :], 1.0)
            to_rows(xh_nat, IC, ic_aug, [2, 3])
            to_rows(xl_nat, IC, ic_aug, [4])
            to_rows(yh_nat, IC, ic_aug, [5, 6])
            to_rows(yl_nat, IC, ic_aug, [7])

        # ---- main loop ----
        with (
            tcx.tile_pool(name="psum_l", bufs=2, space="PSUM") as pl_pool,
            tcx.tile_pool(name="psum_nd", bufs=2, space="PSUM") as nd_pool,
            tcx.tile_pool(name="w", bufs=3) as w_pool,
        ):
            for oc in range(OC):
                nd = nd_pool.tile([128, F], F32)
                for icc in range(IC):
                    pl = pl_pool.tile([128, F], F32)
                    lhsT1 = ic_aug[:, icc * 128:(icc + 1) * 128]
                    for sub in range(NSUB):
                        nc.tensor.matmul(
                            pl[:, sub * 512:(sub + 1) * 512],
                            lhsT1,
                            tc_aug[:, oc * F + sub * 512: oc * F + (sub + 1) * 512],
                            start=True,
                            stop=True,
                        )
                    w = w_pool.tile([128, F], F16)
                    nc.scalar.activation(
                        w[:, :],
                        pl[:, :],
                        AF.Exp,
                        bias=bias_nat[:, icc:icc + 1],
                        scale=2.0 * GAMMA,
                    )
                    for sub in range(NSUB):
                        nc.tensor.matmul(
                            nd[:, sub * 512:(sub + 1) * 512],
                            vo_nat[:, 128 * icc:128 * icc + 128],
                            w[:, sub * 512:(sub + 1) * 512],
                            start=(icc == 0),
                            stop=(icc == IC - 1),
                        )
                nc.vector.tensor_copy(num_row[0:1, oc * F:(oc + 1) * F], nd[0:1, :])
                nc.vector.tensor_copy(den_row[0:1, oc * F:(oc + 1) * F], nd[32:33, :])

        # ---- tail: divide + broadcast in [128, L] layout ----
        with tcx.tile_pool(name="tail", bufs=1) as tail_pool:
            num_pm = tail_pool.tile([128, L], F32)
            den_pm = tail_pool.tile([128, L], F32)
            nc.sync.dma_start(
                out=num_pm[:, :],
                in_=num_row[0:1, :].rearrange("r (p k) -> r p k", p=128),
            )
            nc.sync.dma_start(
                out=den_pm[:, :],
                in_=den_row[0:1, :].rearrange("r (p k) -> r p k", p=128),
            )
            recip = tail_pool.tile([128, L], F32)
            nc.vector.tensor_scalar_add(recip[:, :], den_pm[:, :], EPS * WSCALE)
            nc.vector.reciprocal(recip[:, :], recip[:, :])
            interp_pm = tail_pool.tile([128, L], F32)
            nc.vector.tensor_mul(interp_pm[:, :], num_pm[:, :], recip[:, :])

            rep_sb = tail_pool.tile([128, 4 * L], F32)
            rep3 = rep_sb.rearrange("p (k t) -> p k t", t=4)
            for t in range(4):
                nc.vector.tensor_copy(rep3[:, :, t], interp_pm[:, :])
            for si in range(s):
                nc.sync.dma_start(
                    out=out_h[:][si].rearrange("o t -> (o t)").rearrange(
                        "(p j) -> p j", p=128),
                    in_=rep_sb[:, :],
                )


@lru_cache(maxsize=2)
def build_nc(n_in=N_IN, n_out=N_OUT, s=S, F=1024):
    nc = bacc.Bacc("TRN2", target_bir_lowering=False, debug=False)
    dat = nc.dram_tensor("dat", [T_IN, n_in, V_IN], F32, kind="ExternalInput")
    ic_h = nc.dram_tensor("ic", [n_in, 2], F32, kind="ExternalInput")
    tc_h = nc.dram_tensor("tc", [n_out, 2], F32, kind="ExternalInput")
    out_h = nc.dram_tensor("out", [s, n_out, T_OUT], F32, kind="ExternalOutput")
    with tile.TileContext(nc) as tcx:
        build_kernel(tcx, dat, ic_h, tc_h, out_h, n_in, n_out, s, F=F)
    nc.compile()
    return nc


def _run(input_data, input_coords, target_coords, n_samples, trace=False):
    n_samples = int(n_samples)
    assert n_samples == S, f"kernel compiled for n_samples={S}, got {n_samples}"
    assert input_data.shape == (B, T_IN, N_IN, V_IN)
    nc = build_nc()
    in_maps = [
        {
            "dat": np.ascontiguousarray(input_data[b], dtype=np.float32),
            "ic": np.ascontiguousarray(input_coords[b], dtype=np.float32),
            "tc": np.ascontiguousarray(target_coords[b], dtype=np.float32),
        }
        for b in range(B)
    ]
    res = run_bass_kernel_spmd(nc, in_maps, list(range(B)), trace=trace)
    out = np.stack([res.results[b]["out"] for b in range(B)], axis=0)
    return out, res


def kernel(input_data, input_coords, target_coords, n_samples):
    out, _ = _run(
        np.asarray(input_data),
        np.asarray(input_coords),
        np.asarray(target_coords),
        n_samples,
    )
    return out
